# revision 30
# baseline (speedup 1.0000x reference)
"""Trainium2 Bass kernel for a pre-LN transformer decode layer.

nn_DecodeLayer: x [4, 2048, 1024] f32, 16 heads, causal attention, 4x MLP.

Sharding: 8 cores = 4 batch x 2 query-shards. Core c handles batch c%4 and
query tiles {2j + c//4 : j in 0..7}. The host PERMUTES each core's sequence
(swapping adjacent 128-tile pairs for parity-1 cores) so that every core's
own query tiles sit at EVEN local positions; the kernel is a single uniform
SPMD program and all per-core differences (x layout, causal masks) are data.

Precision plan (rel-err budget 2e-2, sims at ~4e-3): the attention side
(LN1 stats, K/Q/V, probs@V, proj) runs fp8 e4m3 with DoubleRow matmuls
(contract 256 per instruction, ~1.4x tensor throughput); scores K^T@Q and
the whole FFN stay bf16 (fp8 FFN alone sims at 2.7e-2 -- over budget).
Causal masking is a multiplicative DVE op on the exp'd probabilities
(saves 256 PE mask-matmuls + weight loads). Softmax denominators come free
via a ones-column appended to V; 1/sqrt(d) and LN gains fold into weights.
"""

import sys

for _p in ("/opt/trn_rl_repo",):
    if _p not in sys.path:
        sys.path.insert(0, _p)

import numpy as np
import ml_dtypes

import concourse.bass as bass
import concourse.tile as tile
from concourse import bacc, mybir
from concourse.bass_utils import run_bass_kernel_spmd

F32 = mybir.dt.float32
BF16 = mybir.dt.bfloat16
F8 = mybir.dt.float8e4
DR = mybir.MatmulPerfMode.DoubleRow

E = 1024          # d_model
S = 2048          # sequence length
BATCH = 4
NH = 16           # heads
HD = 64           # head dim
P = 128
ET = E // P       # 8 e-tiles
EP = ET // 2      # 4 e-tile PAIRS (DoubleRow contracts 256)
QC = 1024         # queries per core
NKT = S // P      # 16 key tiles
NKP = NKT // 2    # 8 key-tile pairs
FF = 4 * E        # 4096
HT = FF // P      # 32 hidden tiles
N_CORES = 8
EPS = 1e-5


def _segs(q0):
    """Split [q0, 1024) at the PSUM bank boundary (512 f32 cols)."""
    segs = []
    if q0 < 512:
        segs.append((q0, 512 - q0))
    segs.append((max(512, q0), QC - max(512, q0)))
    return segs


def build_program(repeat=1, debug=False):
    nc = bacc.Bacc("TRN2", num_devices=N_CORES)

    d = {}
    def din(name, shape, dtype):
        d[name] = nc.dram_tensor(name, shape, dtype, kind="ExternalInput").ap()

    din("x_full_8", [E, S], F8)        # x[b].T, fp8, seq tiles permuted
    din("x_chunk", [E, QC], F32)       # this core's query rows (f32 residual)
    din("wq", [E, E], F8)              # ln1_g-folded, /8-folded
    din("wk", [E, E], F8)
    din("wv", [E, E], F8)
    din("wproj", [E, E], F8)
    din("wfc", [E, FF], BF16)          # ln2_g-folded
    din("wfc2", [FF, E], BF16)
    din("bq", [E], F32)
    din("bk", [E], F32)
    din("bv", [E], F32)
    din("bproj", [E], F32)
    din("bfc", [FF], F32)
    din("bfc2", [E], F32)
    din("mask", [P, 2 * P], BF16)      # MULTIPLICATIVE mask (1 / 0)
    out_ap = nc.dram_tensor("out", [E, QC], F32, kind="ExternalOutput").ap()
    dbg = {}
    if debug:
        for nm, shape, dt_ in (("dbg_xn", [E, S], F8), ("dbg_kt", [E, S], BF16),
                               ("dbg_qt", [E, QC], BF16),
                               ("dbg_va", [S, NH * (HD + 1)], F8),
                               ("dbg_attnT", [E, QC], F8),
                               ("dbg_x2", [E, QC], F32),
                               ("dbg_xn2", [E, QC], BF16)):
            dbg[nm] = nc.dram_tensor(nm, shape, dt_, kind="ExternalOutput").ap()

    with tile.TileContext(nc) as tc:
        if repeat == 1:
            _emit(nc, tc, d, out_ap, dbg)
        else:
            with tc.For_i(0, repeat, 1):
                _emit(nc, tc, d, out_ap, dbg)

    nc.compile()
    return nc


def _emit(nc, tc, d, out_ap, dbg=None):
    dbg = dbg or {}
    A = mybir.ActivationFunctionType
    O = mybir.AluOpType
    import contextlib
    ctx = contextlib.ExitStack()
    with ctx:
        # --- long-lived pools ---
        pconst = ctx.enter_context(tc.tile_pool(name="pconst", bufs=1))
        pbig = ctx.enter_context(tc.tile_pool(name="pbig", bufs=1))
        prows = ctx.enter_context(tc.tile_pool(name="prows", bufs=1))
        pbc = ctx.enter_context(tc.tile_pool(name="pbc", bufs=2))

        # --- constants ---
        ones8 = pconst.tile([P, 2, P], F8, tag="ones8")
        nc.vector.memset(ones8, 1.0)
        ones_bf = pconst.tile([P, P], BF16, tag="onesb")
        nc.vector.memset(ones_bf, 1.0)
        eps_t = pconst.tile([P, 1], F32, tag="eps")
        nc.vector.memset(eps_t, EPS)
        mask_sb = pconst.tile([P, 2 * P], BF16, tag="mask")
        nc.sync.dma_start(out=mask_sb, in_=d["mask"])
        mask2v = mask_sb.rearrange("p (s c) -> p s c", c=P)

        def bias_cols(name, n_tiles):
            t = pconst.tile([P, n_tiles], F32, tag=f"b_{name}", name=f"b_{name}")
            nc.sync.dma_start(out=t, in_=d[name].rearrange("(t p) -> p t", p=P))
            return t

        bq_sb = bias_cols("bq", ET)
        bk_sb = bias_cols("bk", ET)
        bproj_sb = bias_cols("bproj", ET)
        bfc2_sb = bias_cols("bfc2", ET)
        bfc_sb = bias_cols("bfc", HT)

        # big persistent tiles. X0/X1 are flat 16KB regions: the first 8KB
        # holds the raw fp8 x half (contiguous, so the input DMA is fully
        # dense), the next 4KB the packed query columns; later the whole
        # region is reused as bf16 FFN-hidden storage (H_b/H_c).
        x8f_h = [pbig.tile([P, 2 * ET * 1024], F8, tag=f"X{i}", name=f"x8{i}")
                 for i in range(2)]
        x8_h = [t[:, 0:ET * 1024].rearrange("p (t c) -> p t c", c=1024)
                for t in x8f_h]
        x8q_h = [t[:, ET * 1024:ET * 1024 + ET * 512]
                 .rearrange("p (t c) -> p t c", c=512) for t in x8f_h]
        xn8_h = [pbig.tile([P, ET, 1024], F8, tag=f"XN{i}", name=f"xn8{i}")
                 for i in range(2)]
        attnT = pbig.tile([P, ET, QC], F8, tag="T2")
        KT = pbig.tile([P, ET, S], BF16, tag="T3")       # -> H_a
        QT = pbig.tile([P, ET, QC], BF16, tag="T4")      # -> xn2
        VA = pbig.tile([P, NKT, NH * (HD + 1)], F8, tag="T5")
        xc = pbig.tile([P, ET, QC], F32, tag="XC", name="xc")  # residual rows

        # V weights resident (DMA overlaps LN1)
        pwv_cm = tc.tile_pool(name="pwv", bufs=1)
        pwv = pwv_cm.__enter__()
        wvf = pwv.tile([P, ET, E], F8, tag="wvf")
        for et in range(ET):
            nc.sync.dma_start(out=wvf[:, et, :],
                              in_=d["wv"][et * P:(et + 1) * P, :])
        bvrow = prows.tile([1, E], F32, tag="rows", name="rows")
        nc.sync.dma_start(out=bvrow, in_=d["bv"].rearrange("(o n) -> o n", o=1))
        bvrow_bf = prows.tile([1, E], BF16, tag="rows_bf", name="rows_bf")
        nc.gpsimd.tensor_copy(bvrow_bf, bvrow)
        bvb = pconst.tile([P, E], BF16, tag="bvb")
        nc.gpsimd.partition_broadcast(bvb, bvrow_bf)

        # ---- phase 1: LN1 over the full (permuted) sequence, fp8 in/out ----
        with tc.tile_pool(name="pst1", bufs=2, space="PSUM") as pst:
            for h in range(2):
                for et in range(ET):
                    nc.sync.dma_start(
                        out=x8_h[h][:, et, :],
                        in_=d["x_full_8"][et * P:(et + 1) * P,
                                          h * 1024:(h + 1) * 1024])
            stats = []
            for h in range(2):
                ps_x = pst.tile([P, 1024], F32, tag="st_x", name="st_x")
                ps_q = pst.tile([P, 1024], F32, tag="st_q", name="st_q")
                for tp in range(EP):
                    sq8 = pbc.tile([P, 2, 1024], F8, tag="sq", bufs=1,
                                   name="sq")
                    for sl in range(2):
                        et = 2 * tp + sl
                        xb = x8_h[h][:, et, :]
                        if et % 4 == 3:
                            nc.vector.tensor_mul(sq8[:, sl, :], xb, xb)
                        else:
                            nc.scalar.activation(sq8[:, sl, :], xb, A.Square)
                    for c0 in (0, 512):
                        sl2 = slice(c0, c0 + 512)
                        nc.tensor.matmul(
                            ps_x[:, sl2], ones8,
                            x8_h[h][:, 2 * tp:2 * tp + 2, c0:c0 + 512],
                            start=(tp == 0), stop=(tp == EP - 1), perf_mode=DR)
                        nc.tensor.matmul(
                            ps_q[:, sl2], ones8, sq8[:, :, c0:c0 + 512],
                            start=(tp == 0), stop=(tp == EP - 1), perf_mode=DR)
                stats.append((ps_x, ps_q))
            for h in range(2):
                ps_x, ps_q = stats[h]
                m_bf = pbc.tile([P, 1024], BF16, tag="mbf", bufs=1, name="mbf")
                nc.scalar.activation(m_bf, ps_x, A.Copy, scale=1.0 / E)
                e2 = pbc.tile([P, 1024], F32, tag="e2", bufs=1, name="e2")
                nc.scalar.activation(e2, ps_q, A.Copy, scale=1.0 / E)
                m2 = pbc.tile([P, 1024], F32, tag="m2", bufs=1, name="m2")
                nc.scalar.activation(m2, m_bf, A.Square)
                nc.vector.tensor_sub(e2, e2, m2)
                nc.scalar.activation(e2, e2, A.Sqrt, bias=eps_t)
                rstd = pbc.tile([P, 1024], F32, tag="m2", bufs=1, name="m2r")
                nc.vector.reciprocal_approx_fast(rstd, e2)
                rstd_bf = pbc.tile([P, 1024], BF16, tag="rbf", bufs=1,
                                   name="rbf")
                nc.vector.tensor_copy(rstd_bf, rstd)
                for et in range(ET):
                    xb = x8_h[h][:, et, :]
                    tmp = pbc.tile([P, 1024], BF16, tag="tmp", bufs=1,
                                   name="tmp")
                    nc.vector.tensor_sub(tmp, xb, m_bf)
                    nc.vector.tensor_mul(xn8_h[h][:, et, :], tmp, rstd_bf)
                    # packed copy of this core's own (even-local) query cols
                    # into the q-pack section of the flat x8 region
                    nc.scalar.activation(
                        x8q_h[h][:, et, :]
                        .rearrange("p (a b) -> p a b", b=P),
                        xn8_h[h][:, et, :]
                        .rearrange("p (a b) -> p a b", b=256)[:, :, 0:P],
                        A.Copy)

        if "dbg_xn" in dbg:
            for h in range(2):
                nc.sync.dma_start(
                    out=dbg["dbg_xn"][:, h * 1024:(h + 1) * 1024]
                    .rearrange("(t p) c -> p t c", p=P), in_=xn8_h[h])

        # ---- phase 2: V (natural layout + ones cols for denominators) ----
        va_hview = VA.rearrange("p t (h c) -> p t h c", c=HD + 1)
        nc.gpsimd.memset(va_hview[:, :, :, HD:HD + 1], 1.0)
        with tc.tile_pool(name="ppv", bufs=3, space="PSUM") as ppv:
            for t in range(NKT):
                xn_src = xn8_h[t // 8]
                for vh in range(2):
                    hbase = vh * (NH // 2)
                    ps = ppv.tile([P, 512], F32, tag="mm", name="mm")
                    for tp in range(EP):
                        nc.tensor.matmul(
                            ps,
                            xn_src[:, 2 * tp:2 * tp + 2,
                                   (t % 8) * P:(t % 8 + 1) * P],
                            wvf[:, 2 * tp:2 * tp + 2,
                                vh * 512:(vh + 1) * 512],
                            start=(tp == 0), stop=(tp == EP - 1), perf_mode=DR)
                    va_v = va_hview[:, t, hbase:hbase + 8, :]
                    nc.vector.tensor_add(
                        va_v[:, :, 0:HD],
                        ps.rearrange("p (h c) -> p h c", c=HD),
                        bvb[:, vh * 512:(vh + 1) * 512]
                        .rearrange("p (h c) -> p h c", c=HD))

        pwv_cm.__exit__(None, None, None)
        if "dbg_va" in dbg:
            nc.sync.dma_start(
                out=dbg["dbg_va"].rearrange("(t p) c -> p t c", p=P), in_=VA)

        # proj weights + residual rows; DMA here so they overlap attention
        pwp = ctx.enter_context(tc.tile_pool(name="pwp", bufs=1))
        wpf = pwp.tile([P, ET, E], F8, tag="wpf")
        for et in range(ET):
            nc.sync.dma_start(out=wpf[:, et, :],
                              in_=d["wproj"][et * P:(et + 1) * P, :])
        for et in range(ET):
            nc.sync.dma_start(out=xc[:, et, :],
                              in_=d["x_chunk"][et * P:(et + 1) * P, :])

        # ---- phase 3: K/Q + attention, software-pipelined ----
        # scores(b+1) is emitted BEFORE attnV(m-1); each pair's attnV is
        # delayed by one pair so the PE never waits on the exp/mask chain.
        # K/Q matmuls drain one at a time into the exp-latency gaps of the
        # b-loop -- this filler also keeps PE utilization high enough that
        # HAM doesn't clock the tensor engine down mid-attention.
        with tc.tile_pool(name="pwk", bufs=2) as pwk, \
             tc.tile_pool(name="pprobs", bufs=3) as pprobs, \
             tc.tile_pool(name="prb", bufs=2) as prb, \
             tc.tile_pool(name="psc", bufs=2, space="PSUM") as psc, \
             tc.tile_pool(name="pkq", bufs=2, space="PSUM") as pkq, \
             tc.tile_pool(name="ppO", bufs=1, space="PSUM") as ppO:

            def make_kq_units(kd):
                """Per-matmul emission units for kd's 6 K/Q chunks."""
                wtk = pwk.tile([P, ET, P], F8, tag="wck", name="wck")
                nc.sync.dma_start(
                    out=wtk, in_=d["wk"][:, kd * P:(kd + 1) * P]
                    .rearrange("(t p) c -> p t c", p=P))
                wtq = pwk.tile([P, ET, P], F8, tag="wcq", name="wcq")
                nc.sync.dma_start(
                    out=wtq, in_=d["wq"][:, kd * P:(kd + 1) * P]
                    .rearrange("(t p) c -> p t c", p=P))
                units = []
                chunks = [("k", c0, wtk) for c0 in range(0, S, 512)] + \
                         [("q", c0, wtq) for c0 in (0, 512)]
                for (kind, c0, wt) in chunks:
                    st = {}
                    def mm(tp, kind=kind, c0=c0, wt=wt, st=st):
                        if "ps" not in st:
                            st["ps"] = pkq.tile([P, 512], F32, tag="mm",
                                                name="mm")
                        if kind == "k":
                            src = xn8_h[c0 // 1024][
                                :, 2 * tp:2 * tp + 2,
                                c0 % 1024:c0 % 1024 + 512]
                        else:
                            src = x8q_h[c0 // 512][:, 2 * tp:2 * tp + 2, :]
                        nc.tensor.matmul(st["ps"], wt[:, 2 * tp:2 * tp + 2, :],
                                         src, start=(tp == 0),
                                         stop=(tp == EP - 1), perf_mode=DR,
                                         skip_group_check=True)
                    def epi(kind=kind, c0=c0, kd=kd, st=st):
                        dstT, bcol = ((KT, bk_sb) if kind == "k"
                                      else (QT, bq_sb))
                        nc.vector.tensor_scalar(
                            dstT[:, kd, c0:c0 + 512], st["ps"],
                            bcol[:, kd:kd + 1], None, op0=O.add)
                    units += [lambda tp=tp, mm=mm: mm(tp) for tp in range(EP)]
                    units.append(epi)
                return units

            def emit_head(h, fill):
                kdt, off = h // 2, (h % 2) * HD
                psO = ppO.tile([HD + 1, QC], F32, tag="psO", name="psO")

                def emit_scores(b):
                    q0 = (b // 2) * P
                    base = (q0 // 512) * 512
                    ps = psc.tile([P, QC], F32, tag="sc", name="sc")
                    for (s0, ln) in _segs(q0):
                        nc.tensor.matmul(
                            ps[:, s0 - base:s0 - base + ln],
                            KT[off:off + HD, kdt, b * P:(b + 1) * P],
                            QT[off:off + HD, kdt, s0:s0 + ln],
                            start=True, stop=True,
                            skip_group_check=True)
                    return ps

                def make_attnV(m, probs2):
                    def attnV():
                        q0 = m * P
                        for (s0, ln) in _segs(q0):
                            last = 3 if s0 < 512 else NKP - 1
                            nc.tensor.matmul(
                                psO[:, s0:s0 + ln],
                                VA[:, 2 * m:2 * m + 2,
                                   h * (HD + 1):(h + 1) * (HD + 1)],
                                probs2[:, :, s0 - q0:s0 - q0 + ln],
                                start=(m == 0), stop=(m == last),
                                perf_mode=DR, skip_group_check=True)
                    return attnV

                # per-b scores pipeline (PSUM WAR slack of one b) feeding
                # per-pair fp8 probs; each pair's attnV is delayed by one
                # pair so the PE never waits on the exp/mask chain.
                sc_next = emit_scores(0)
                probs2 = None
                attnV_prev = None
                for b in range(NKT):
                    m, slot = divmod(b, 2)
                    q0 = m * P
                    qlen = QC - q0
                    base = (q0 // 512) * 512
                    if slot == 0:
                        probs2 = pprobs.tile([P, 2, QC], F8, tag="probs",
                                             name="probs")
                    ps = sc_next
                    nc.scalar.activation(
                        probs2[:, slot, 0:qlen],
                        ps[:, q0 - base:q0 - base + qlen], A.Exp)
                    if b + 1 < NKT:
                        sc_next = emit_scores(b + 1)
                    for _ in range(2):
                        if fill:
                            fill.pop(0)()
                    if slot == 1:
                        # multiplicative causal mask, diagonal 128-col block
                        nc.vector.tensor_mul(probs2[:, :, 0:P],
                                             probs2[:, :, 0:P], mask2v)
                        if attnV_prev is not None:
                            attnV_prev()
                        attnV_prev = make_attnV(m, probs2)
                attnV_prev()
                # psO evacuation: the denominator row goes through an ACT
                # copy (DVE reciprocal straight from PSUM at partition
                # offset 64 misreads on HW); the wide [64,1024] read is a
                # DVE mul so ACT -- the attention bottleneck -- stays free.
                srow = prows.tile([1, QC], F32, tag="rows", name="rows")
                nc.scalar.activation(srow, psO[HD:HD + 1, :], A.Copy)
                srow2 = prows.tile([1, QC], F32, tag="rows2", name="rows2")
                nc.vector.reciprocal_approx_fast(srow2, srow)
                srow_bf = prows.tile([1, QC], BF16, tag="rows_bf",
                                     name="rows_bf")
                nc.vector.tensor_copy(srow_bf, srow2)
                rb = prb.tile([HD, QC], BF16, tag="rb", name="rb")
                nc.gpsimd.partition_broadcast(rb, srow_bf)
                nc.vector.tensor_mul(attnT[off:off + HD, kdt, :],
                                     psO[0:HD, :], rb)

            for u in make_kq_units(0):
                u()
            pending = []
            for kd in range(ET):
                if kd + 1 < ET:
                    pending = make_kq_units(kd + 1)
                for h in (2 * kd, 2 * kd + 1):
                    emit_head(h, pending)
                for u in pending:
                    u()
                pending = []

        if "dbg_kt" in dbg:
            nc.sync.dma_start(
                out=dbg["dbg_kt"].rearrange("(t p) c -> p t c", p=P), in_=KT)
        if "dbg_qt" in dbg:
            nc.sync.dma_start(
                out=dbg["dbg_qt"].rearrange("(t p) c -> p t c", p=P), in_=QT)
        if "dbg_attnT" in dbg:
            nc.sync.dma_start(
                out=dbg["dbg_attnT"].rearrange("(t p) c -> p t c", p=P),
                in_=attnT)

        # ---- phase 4+5a: proj + residual -> xc (in place); LN2 -> xn2;
        # fc1 for the finished 512-query chunk follows immediately so the PE
        # stays warm through the LN2 epilogue chain. ----
        xn2 = pbig.tile([P, ET, QC], BF16, tag="T4", name="xn2")
        H_a = pbig.tile([P, 16, 1024], BF16, tag="T3", name="H_a")
        H_b = pbig.tile([P, ET, 1024], BF16, tag="X0", name="H_b")
        H_c = pbig.tile([P, ET, 1024], BF16, tag="X1", name="H_c")

        def H(ht):
            if ht < 16:
                return H_a[:, ht, :]
            if ht < 24:
                return H_b[:, ht - 16, :]
            return H_c[:, ht - 24, :]

        def layernorm_bf(dst_for_et, xsrc_for_et, pst, half_w):
            ps_x = pst.tile([P, half_w], F32, tag="st_x", name="st_x")
            ps_q = pst.tile([P, half_w], F32, tag="st_q", name="st_q")
            for et in range(ET):
                xb = xsrc_for_et(et)
                sq = pbc.tile([P, half_w], BF16, tag="sqb", name="sqb")
                if et % 4 == 3:
                    nc.vector.tensor_mul(sq, xb, xb)
                else:
                    nc.scalar.activation(sq, xb, A.Square)
                nc.tensor.matmul(ps_x, ones_bf, xb,
                                 start=(et == 0), stop=(et == ET - 1))
                nc.tensor.matmul(ps_q, ones_bf, sq,
                                 start=(et == 0), stop=(et == ET - 1))
            m_bf = pbc.tile([P, half_w], BF16, tag="mbf", bufs=1, name="mbf")
            nc.scalar.activation(m_bf, ps_x, A.Copy, scale=1.0 / E)
            e2 = pbc.tile([P, half_w], F32, tag="e2", bufs=1, name="e2")
            nc.scalar.activation(e2, ps_q, A.Copy, scale=1.0 / E)
            m2 = pbc.tile([P, half_w], F32, tag="m2", bufs=1, name="m2")
            nc.scalar.activation(m2, m_bf, A.Square)
            nc.vector.tensor_sub(e2, e2, m2)
            nc.scalar.activation(e2, e2, A.Sqrt, bias=eps_t)
            rstd = pbc.tile([P, half_w], F32, tag="m2", bufs=1, name="m2r")
            nc.vector.reciprocal_approx_fast(rstd, e2)
            rstd_bf = pbc.tile([P, half_w], BF16, tag="rbf", bufs=1,
                               name="rbf")
            nc.vector.tensor_copy(rstd_bf, rstd)
            for et in range(ET):
                xb = xsrc_for_et(et)
                dst = dst_for_et(et)
                nc.vector.tensor_sub(dst, xb, m_bf)
                nc.vector.tensor_mul(dst, dst, rstd_bf)

        # fc1 weight double-buffer lives in the dead xn8 regions; x2bf in VA's
        wt_fc1 = [pbig.tile([P, ET, 512], BF16, tag=f"XN{i}", name=f"wfc1{i}")
                  for i in range(2)]
        x2full = pbig.tile([P, ET, 1024], BF16, tag="T5", name="x2full")
        with tc.tile_pool(name="pppr", bufs=1, space="PSUM") as ppp, \
             tc.tile_pool(name="pst2", bufs=2, space="PSUM") as pst2, \
             tc.tile_pool(name="ppff", bufs=2, space="PSUM") as ppf:
            # proj(0), LN2(0)-stats, proj(512), LN2(512)-stats, fc1(0),
            # fc1(512): each LN2 epilogue chain overlaps the next PE block.
            for c0 in (0, 512):
                for et in range(ET):
                    ps = ppp.tile([P, 512], F32, tag="mm", bufs=2, name="mm")
                    for tp in range(EP):
                        nc.tensor.matmul(
                            ps, wpf[:, 2 * tp:2 * tp + 2, et * P:(et + 1) * P],
                            attnT[:, 2 * tp:2 * tp + 2, c0:c0 + 512],
                            start=(tp == 0), stop=(tp == EP - 1), perf_mode=DR)
                    nc.vector.scalar_tensor_tensor(
                        xc[:, et, c0:c0 + 512], in0=ps,
                        scalar=bproj_sb[:, et:et + 1],
                        in1=xc[:, et, c0:c0 + 512],
                        op0=O.add, op1=O.add)
                    nc.scalar.activation(x2full[:, et, c0:c0 + 512],
                                         xc[:, et, c0:c0 + 512], A.Copy)
                layernorm_bf(lambda et, c0=c0: xn2[:, et, c0:c0 + 512],
                             lambda et, c0=c0: x2full[:, et, c0:c0 + 512],
                             pst2, half_w=512)
            # fc1 (weights re-streamed per chunk; the extra 8MB of DMA
            # hides under the matmuls)
            for c0 in (0, 512):
                for hg in range(8):
                    wt = wt_fc1[hg % 2]
                    nc.sync.dma_start(
                        out=wt,
                        in_=d["wfc"][:, hg * 512:(hg + 1) * 512]
                        .rearrange("(t p) c -> p t c", p=P))
                    for h4 in range(4):
                        ht = hg * 4 + h4
                        psA = ppf.tile([P, 512], F32, tag="mmA", name="mmA")
                        for et in range(ET):
                            nc.tensor.matmul(
                                psA, wt[:, et, h4 * P:(h4 + 1) * P],
                                xn2[:, et, c0:c0 + 512],
                                start=(et == 0), stop=(et == ET - 1))
                        nc.scalar.activation(H(ht)[:, c0:c0 + 512], psA,
                                             A.Gelu, bias=bfc_sb[:, ht:ht + 1])

        if "dbg_x2" in dbg:
            nc.sync.dma_start(
                out=dbg["dbg_x2"].rearrange("(t p) c -> p t c", p=P), in_=xc)
        if "dbg_xn2" in dbg:
            nc.sync.dma_start(
                out=dbg["dbg_xn2"].rearrange("(t p) c -> p t c", p=P), in_=xn2)

        # ---- phase 5b: fc2 in 4 output groups (small output-DMA tail) ----
        og2 = pbig.tile([P, 2, 512], F32, tag="T5", name="og2")
        n_og = [0]
        with tc.tile_pool(name="pwf2", bufs=6) as pwf2, \
             tc.tile_pool(name="ppff2", bufs=2, space="PSUM") as ppf2:
            for eg in range(4):
                psY = [ppf2.tile([P, QC], F32, tag=f"psY{i}", name=f"psY{i}")
                       for i in range(2)]
                for ht in range(HT):
                    wt = pwf2.tile([P, 256], BF16, tag="wfc2", name="wfc2")
                    nc.sync.dma_start(
                        out=wt,
                        in_=d["wfc2"][ht * P:(ht + 1) * P,
                                      eg * 256:(eg + 1) * 256])
                    for e2_ in range(2):
                        for qch in (0, 512):
                            nc.tensor.matmul(
                                psY[e2_][:, qch:qch + 512],
                                wt[:, e2_ * P:(e2_ + 1) * P],
                                H(ht)[:, qch:qch + 512],
                                start=(ht == 0), stop=(ht == HT - 1))
                for e2_ in range(2):
                    et = eg * 2 + e2_
                    for qch in (0, 512):
                        og = og2[:, n_og[0] % 2, :]
                        n_og[0] += 1
                        nc.vector.scalar_tensor_tensor(
                            og, in0=psY[e2_][:, qch:qch + 512],
                            scalar=bfc2_sb[:, et:et + 1],
                            in1=xc[:, et, qch:qch + 512],
                            op0=O.add, op1=O.add)
                        nc.sync.dma_start(
                            out=out_ap[et * P:(et + 1) * P, qch:qch + 512],
                            in_=og)

# ---------------------------------------------------------------------------
# host side
# ---------------------------------------------------------------------------

_PROG_CACHE = {}


def get_program(repeat=1):
    key = repeat
    if key not in _PROG_CACHE:
        _PROG_CACHE[key] = build_program(repeat)
    return _PROG_CACHE[key]


def _f8(a):
    return np.ascontiguousarray(
        np.clip(np.asarray(a, np.float32), -240.0, 240.0)
        .astype(ml_dtypes.float8_e4m3))


def prep_in_maps(x, ln1_g, ln1_b, w_attn, b_attn, w_proj, b_proj,
                 ln2_g, ln2_b, w_fc, b_fc, w_fc2, b_fc2):
    f32 = np.float32
    bf = ml_dtypes.bfloat16
    x = np.asarray(x, f32)
    g1 = np.asarray(ln1_g, f32)[:, None]
    wq = (g1 * np.asarray(w_attn[:, 0:E], f32)) / 8.0
    wk = g1 * np.asarray(w_attn[:, E:2 * E], f32)
    wv = g1 * np.asarray(w_attn[:, 2 * E:3 * E], f32)
    bq = (np.asarray(w_attn[:, 0:E], f32).T @ np.asarray(ln1_b, f32)
          + np.asarray(b_attn[0:E], f32)) / 8.0
    bk = (np.asarray(w_attn[:, E:2 * E], f32).T @ np.asarray(ln1_b, f32)
          + np.asarray(b_attn[E:2 * E], f32))
    bv = (np.asarray(w_attn[:, 2 * E:3 * E], f32).T @ np.asarray(ln1_b, f32)
          + np.asarray(b_attn[2 * E:3 * E], f32))
    g2 = np.asarray(ln2_g, f32)[:, None]
    wfc = g2 * np.asarray(w_fc, f32)
    bfc = np.asarray(w_fc, f32).T @ np.asarray(ln2_b, f32) + np.asarray(b_fc, f32)

    shared = {
        "wq": _f8(wq),
        "wk": _f8(wk),
        "wv": _f8(wv),
        "wproj": _f8(np.asarray(w_proj, f32)),
        "wfc": np.ascontiguousarray(wfc.astype(bf)),
        "wfc2": np.ascontiguousarray(np.asarray(w_fc2, f32).astype(bf)),
        "bq": np.ascontiguousarray(bq.astype(f32)),
        "bk": np.ascontiguousarray(bk.astype(f32)),
        "bv": np.ascontiguousarray(bv.astype(f32)),
        "bproj": np.ascontiguousarray(np.asarray(b_proj, f32)),
        "bfc": np.ascontiguousarray(bfc.astype(f32)),
        "bfc2": np.ascontiguousarray(np.asarray(b_fc2, f32)),
    }

    # MULTIPLICATIVE masks (bf16, 1 = visible / 0 = masked): [:, :128] is the
    # even-local-key-tile (diagonal) block, [:, 128:] the odd one. With the
    # per-core permutation, even b is always the diagonal block; odd b is
    # fully-masked (parity 0) or fully visible (parity 1).
    vis = (np.arange(P)[:, None] <= np.arange(P)[None, :]).astype(np.float32)
    masks = []
    for parity in (0, 1):
        m = np.zeros((P, 2 * P), np.float32)
        m[:, 0:P] = vis
        m[:, P:2 * P] = 0.0 if parity == 0 else 1.0
        masks.append(np.ascontiguousarray(m.astype(bf)))

    # parity-1 cores see the sequence with adjacent 128-tile pairs swapped,
    # so their own query tiles are at even local positions
    perm1 = np.arange(S).reshape(NKT, P)[
        [t ^ 1 for t in range(NKT)]].reshape(-1)

    in_maps = []
    for c in range(N_CORES):
        b, parity = c % BATCH, c // BATCH
        xbt = np.ascontiguousarray(x[b].T)          # [E, S]
        rows = np.concatenate(
            [np.arange(P * (2 * j + parity), P * (2 * j + parity) + P)
             for j in range(8)])
        m = dict(shared)
        xloc = xbt if parity == 0 else xbt[:, perm1]
        m["x_full_8"] = _f8(xloc)
        m["x_chunk"] = np.ascontiguousarray(xbt[:, rows])
        m["mask"] = masks[parity]
        in_maps.append(m)
    return in_maps


def assemble_output(results):
    y = np.empty((BATCH, S, E), np.float32)
    for c in range(N_CORES):
        b, parity = c % BATCH, c // BATCH
        rows = np.concatenate(
            [np.arange(P * (2 * j + parity), P * (2 * j + parity) + P)
             for j in range(8)])
        y[b, rows, :] = results[c]["out"].T
    return y


def kernel(**inputs):
    nc = get_program(1)
    in_maps = prep_in_maps(**inputs)
    res = run_bass_kernel_spmd(nc, in_maps, core_ids=list(range(N_CORES)))
    return assemble_output(res.results)
